# revision 16
# baseline (speedup 1.0000x reference)
"""TRN2 Bass kernel for the ConceptualMambaBlock problem.

Math (reference):
    x: [B=4, T=96, N=512, H=128] f32
    expanded = x @ W_exp.T + b_exp            # [B,T,N,2H]
    primary, gating = split(expanded, 2, -1)
    s_t = 0.9*s_{t-1} + 0.1*gating_t          # EMA along T
    out = (primary * sigmoid(s)) @ W_con.T + b_con

Strategy:
  - Shard (B x N/2) over 8 cores: core c -> batch c//2, node half c%2.
  - Host pre-transposes each core's x shard to [H, N_local, T] so the
    contraction dim H lands on SBUF partitions with fully-contiguous DMA;
    no on-chip transposes anywhere.
  - Per 4-node block (tok = 4*96 = 384 columns, t fastest):
      mm1 (fp32r, full PE rate) -> PSUM gating/primary [o=128, tok]
      gating bias via K=1 accumulate-matmul (weights/bias pre-scaled by 0.1)
      EMA via DVE tensor_tensor_scan: state = mask*state + g  (mask has 0.0
      at each t=0 column, so the 4 node-segments reset exactly)
      sigmoid on ACT; gate-mul + primary bias in one DVE op;
      mm2 (fp32r); output bias via ACT Identity; DMA out.
  - Matmuls are batched by weight across groups of GRP=4 blocks so the PE
    streams N-cycle back-to-back matmuls instead of paying the isolated
    (219+N)-cycle cost on every weight switch.  mm2 of group g-1 is emitted
    inside group g (software pipeline) so the PE never waits on the current
    group's DVE/ACT chain.
  - PSUM: "pg" tag holds the gating tiles (4 banks); "pq" tag is shared by
    the primary (pp) and output (po) tiles (4 banks), whose lifetimes
    alternate.
  - DMA is grouped: one load / one store covers GRP consecutive blocks.
  - Output returned as [H, N_local, T] per core; host transposes back.
"""

import numpy as np

import concourse.bacc as bacc
import concourse.bass as bass  # noqa: F401  (engine types referenced via nc)
import concourse.mybir as mybir
import concourse.tile as tile
from concourse.bass_utils import run_bass_kernel_spmd

F32 = mybir.dt.float32
F32R = mybir.dt.float32r
AF = mybir.ActivationFunctionType
ALU = mybir.AluOpType

B, T, N, H = 4, 96, 512, 128
NCORES = 8
NLOC = N // 2          # 256 nodes per core
TOK = 1024             # tokens per block (2 PSUM banks; scans chained)
NBLK = (NLOC * T) // TOK  # 24 blocks per core

_NC_CACHE = None


def _build():
    nc = bacc.Bacc()

    xt_h = nc.dram_tensor("xt", [H, NBLK, TOK], F32R, kind="ExternalInput")
    wpack_h = nc.dram_tensor("wpack", [H, 3 * H], F32R, kind="ExternalInput")
    bpack_h = nc.dram_tensor("bpack", [H, 4], F32, kind="ExternalInput")
    out_h = nc.dram_tensor("out", [H, NBLK, TOK], F32, kind="ExternalOutput")

    HALF = TOK // 2

    with tile.TileContext(nc) as tc:
        with (
            tc.tile_pool(name="consts", bufs=1) as cp,
            tc.tile_pool(name="io", bufs=3) as io,
            tc.tile_pool(name="mid", bufs=5) as mid,
            tc.tile_pool(name="ps", bufs=2, space="PSUM") as ps,
        ):
            wpack_sb = cp.tile([H, 3 * H], F32R, tag="wpack")
            nc.sync.dma_start(out=wpack_sb[:], in_=wpack_h[:, :])
            bpack_sb = cp.tile([H, 4], F32, tag="bpack")
            nc.sync.dma_start(out=bpack_sb[:], in_=bpack_h[:, :])
            w1p_sb = wpack_sb[:, 0:H]
            w1g_sb = wpack_sb[:, H : 2 * H]
            w2_sb = wpack_sb[:, 2 * H : 3 * H]
            bneg_sb = bpack_sb[:, 0:1]
            bg_sb = bpack_sb[:, 1:2]
            b1p_sb = bpack_sb[:, 2:3]
            b2_sb = bpack_sb[:, 3:4]

            # Three mask variants: block k's first segment boundary falls at
            # column c0 = (-TOK*k) mod T, cycling over {0, 32, 64} with period 3.
            def boundary_cols(c0):
                # number of t=0 columns in [c0, TOK) with stride T
                return (TOK - c0 + T - 1) // T

            mask_sbs = []
            for m in range(3):
                c0 = (-TOK * m) % T
                kk = boundary_cols(c0)
                msk = cp.tile([H, TOK], F32, tag=f"mask{m}", name=f"mask{m}")
                nc.gpsimd.memset(msk[:], 0.9)
                nc.gpsimd.memset(msk[:, c0::T], 0.0)
                mask_sbs.append(msk)

            state = {}

            def fixup_view(pg, k):
                c0 = (-TOK * k) % T
                return pg[:, c0::T]

            def emit_stt(k):
                pp, sg = state[k]["pp"], state[k]["sg"]
                y = mid.tile([H, TOK], F32R, tag="y", name="y")
                nc.vector.scalar_tensor_tensor(
                    out=y[:], in0=pp[:], scalar=b1p_sb, in1=sg[:],
                    op0=ALU.add, op1=ALU.mult,
                )
                state[k]["y"] = y

            def emit_mm2_and_out(k):
                y, ob = state[k]["y"], state[k]["ob"]
                po = ps.tile([H, TOK], F32, tag="pq", name="po", bufs=2)
                nc.tensor.matmul(
                    po[:, 0:HALF], lhsT=w2_sb, rhs=y[:, 0:HALF], start=True, stop=True
                )
                nc.tensor.matmul(
                    po[:, HALF:TOK], lhsT=w2_sb, rhs=y[:, HALF:TOK],
                    start=True, stop=True,
                )
                nc.scalar.activation(
                    ob[:], po[:], AF.Identity, bias=b2_sb, scale=1.0
                )
                nc.sync.dma_start(out=out_h[:, k, :], in_=ob[:])
                del state[k]

            prev_s = None
            for k in range(NBLK):
                xt = io.tile([H, TOK], F32R, tag="xt", name="xt")
                nc.sync.dma_start(out=xt[:], in_=xt_h[:, k, :])
                ob = io.tile([H, TOK], F32, tag="ob", name="ob")
                state[k] = {"ob": ob}

                pg = ps.tile([H, TOK], F32, tag="pg", name="pg", bufs=2)
                nc.tensor.matmul(
                    pg[:, 0:HALF], lhsT=w1g_sb, rhs=xt[:, 0:HALF],
                    start=True, stop=True,
                )
                nc.tensor.matmul(
                    pg[:, HALF:TOK], lhsT=w1g_sb, rhs=xt[:, HALF:TOK],
                    start=True, stop=True,
                )
                # sigma-shift fixup on segment-start columns
                pgc = fixup_view(pg, k)
                nc.scalar.activation(pgc, pgc, AF.Identity, bias=bneg_sb, scale=1.0)

                # DVE: previous block's gate-mul first
                if k - 1 in state and "sg" in state.get(k - 1, {}):
                    emit_stt(k - 1)

                pp = ps.tile([H, TOK], F32, tag="pq", name="pp", bufs=2)
                nc.tensor.matmul(
                    pp[:, 0:HALF], lhsT=w1p_sb, rhs=xt[:, 0:HALF],
                    start=True, stop=True,
                )
                nc.tensor.matmul(
                    pp[:, HALF:TOK], lhsT=w1p_sb, rhs=xt[:, HALF:TOK],
                    start=True, stop=True,
                )
                state[k]["pp"] = pp

                s = mid.tile([H, TOK], F32, tag="s", name="s")
                nc.vector.tensor_tensor_scan(
                    out=s[:], data0=mask_sbs[k % 3][:], data1=pg[:],
                    initial=(0.0 if prev_s is None else prev_s[:, TOK - 1 : TOK]),
                    op0=ALU.mult, op1=ALU.add,
                )
                prev_s = s
                sg = mid.tile([H, TOK], F32, tag="sg", name="sg")
                nc.scalar.activation(sg[:], s[:], AF.Sigmoid, bias=bg_sb, scale=1.0)
                state[k]["sg"] = sg

                if k - 1 in state and "y" in state.get(k - 1, {}):
                    emit_mm2_and_out(k - 1)

            emit_stt(NBLK - 1)
            emit_mm2_and_out(NBLK - 1)

    nc.finalize()
    return nc


def _get_nc():
    global _NC_CACHE
    if _NC_CACHE is None:
        _NC_CACHE = _build()
    return _NC_CACHE


def _in_maps(x, W_exp, b_exp, W_con, b_con):
    wpack = np.concatenate(
        [W_exp[:H, :].T, (0.1 * W_exp[H:, :]).T, W_con.T], axis=1
    ).astype(np.float32)
    wpack = np.ascontiguousarray(wpack)
    bpack = np.stack(
        [-0.9 * b_exp[H:], b_exp[H:], b_exp[:H], b_con], axis=1
    ).astype(np.float32)
    bpack = np.ascontiguousarray(bpack)

    maps = []
    for c in range(NCORES):
        bb, nh = c // 2, c % 2
        xs = x[bb, :, nh * NLOC : (nh + 1) * NLOC, :]  # [T, NLOC, H]
        xT = np.ascontiguousarray(xs.transpose(2, 1, 0)).reshape(H, NBLK, TOK)
        maps.append(
            {
                "xt": xT,
                "wpack": wpack,
                "bpack": bpack,
            }
        )
    return maps


def run_spmd(x, W_exp, b_exp, W_con, b_con, **spmd_kwargs):
    """Run the 8-core kernel; returns (full_output, BassKernelResults)."""
    maps = _in_maps(x, W_exp, b_exp, W_con, b_con)
    res = run_bass_kernel_spmd(
        _get_nc(), maps, core_ids=list(range(NCORES)), **spmd_kwargs
    )
    out = np.empty((B, T, N, H), dtype=np.float32)
    for c in range(NCORES):
        bb, nh = c // 2, c % 2
        oT = res.results[c]["out"].reshape(H, NLOC, T)
        out[bb, :, nh * NLOC : (nh + 1) * NLOC, :] = oT.transpose(2, 1, 0)
    return out, res


def kernel(spatial_temporal_representation, W_exp, b_exp, W_con, b_con):
    out, _ = run_spmd(
        np.asarray(spatial_temporal_representation, dtype=np.float32),
        np.asarray(W_exp, dtype=np.float32),
        np.asarray(b_exp, dtype=np.float32),
        np.asarray(W_con, dtype=np.float32),
        np.asarray(b_con, dtype=np.float32),
    )
    return out
